# revision 1
# baseline (speedup 1.0000x reference)
"""ARAP loss kernel for Trainium2 (8 NeuronCores, SPMD over the vertex axis).

Problem: nn_ArapLoss — per-vertex 6-neighbor gather on a 316x316 grid mesh,
3x3 polar decomposition (via closed-form symmetric eigenanalysis) per vertex,
cotan-weighted edge-residual energy, clamped mean over vertices.

Strategy
--------
- Shard the vertex axis N=99856 across 8 cores (12482 each, padded to
  12544 = 128*98). The adjacency of the grid mesh reduces to K=6 constant
  index offsets {+-1, +-316, +-317}; the host reorganizes the (N, D)
  adjacency into per-offset-class dense arrays and materializes shifted
  windows of `prediction`, so the device does NO gather at all — every
  neighbor access is a dense strided window.
- Device layout: partition = 128 vertex groups, free dim = (batch-quarter,
  98 vertices). Per-vertex constants broadcast along the batch axis with
  stride-0 access patterns.
- R is computed WITHOUT the (catastrophically cancelling) smallest
  eigenvalue: R = A(T2' + d T3') + d cof(A(T2'+T3')), using
  cof(u2 v2^T + u3 v3^T) = det(U)det(V) u1 v1^T and d = sign(det A).
- Output: per-core partial sums [128, 16]; host reduces and divides by N.
"""
import sys

for _p in ("/opt/trn_rl_repo", "/opt/trn_rl_repo/concourse", "/opt/pypackages"):
    if _p not in sys.path:
        sys.path.insert(0, _p)

from contextlib import ExitStack

import numpy as np

import concourse.bass as bass
import concourse.tile as tile
from concourse import bacc, mybir
from concourse.bass_utils import run_bass_kernel_spmd

F32 = mybir.dt.float32
AL = mybir.AluOpType
AF = mybir.ActivationFunctionType

# ---- problem geometry (hardcoded per spec) --------------------------------
B = 16
NV = 99856
NCORES = 8
P = 128
NC_V = NV // NCORES            # 12482 real vertices per core
FQ = 98                        # free-dim vertices per partition
VP = P * FQ                    # 12544 padded vertices per core
BQ = 4                         # batch elements per pass
NQ = B // BQ
STAB = 1000.0
CLIPV = 1e-6                   # 1e-12 * stab^2
LN2 = float(np.log(2.0))
C_SINL = float(2.0 * np.pi / 3.0)
RCLAMP = 1.0 - 1e-6

_nc_cache = {}


# ---------------------------------------------------------------------------
# Host-side preprocessing
# ---------------------------------------------------------------------------

def _build_offset_classes(adj_idx, adj_w, tev_T, tev_w):
    """(N,D) adjacency -> per-offset-class arrays wk (K,N), Wk (K,N,3),
    tk (K,N,3). Padding entries (idx 0 beyond row count) are dropped."""
    N, D = adj_idx.shape
    ar = np.arange(N, dtype=np.int64)
    real = (adj_idx > 0) | (np.arange(D)[None, :] == 0)
    delta = np.asarray(adj_idx, np.int64) - ar[:, None]
    offs = np.unique(delta[real])
    K = len(offs)
    if K > 12:
        raise NotImplementedError(f"too many offset classes: {K}")
    wk = np.zeros((K, N), np.float32)
    Wk = np.zeros((K, N, 3), np.float32)
    tk = np.zeros((K, N, 3), np.float32)
    for k, o in enumerate(offs):
        sel = real & (delta == o)
        n_id, d_id = np.nonzero(sel)
        wk[k, n_id] = adj_w[n_id, d_id]
        Wk[k, n_id] = tev_w[n_id, d_id, :]
        tk[k, n_id] = tev_T[n_id, :, d_id]
    return [int(o) for o in offs], wk, Wk, tk


def _group_offsets(offs, gap=8):
    """Group [0]+offs into consecutive runs; returns (bases, width, win_map)
    where win_map[x] = (g, slot) for x in [0(center)] + offs order."""
    allo = sorted(set([0] + list(offs)))
    groups = [[allo[0]]]
    for o in allo[1:]:
        if o - groups[-1][-1] <= gap:
            groups[-1].append(o)
        else:
            groups.append([o])
    bases = [g[0] for g in groups]
    width = FQ + max(g[-1] - g[0] for g in groups) + 1
    lut = {}
    for gi, g in enumerate(groups):
        for o in g:
            lut[o] = (gi, o - g[0])
    win_map = [lut[0]] + [lut[o] for o in offs]
    return bases, width, win_map


def _host_prepare(pred, offs, wk, Wk, tk):
    """Build per-core input maps: predl [P, B*3*G*GWD] and constl [P, CW*FQ]."""
    K = len(offs)
    bases, GWD, win_map = _group_offsets(offs)
    G = len(bases)
    CW = 3 * K + 3 + 3 * K + K               # Wk(18) WS(3) tk(18) wk(6)
    H = max(max(abs(o) for o in offs), 1)
    padlen = NV + 2 * H + (VP - NC_V) + GWD
    padG = np.zeros((B, 3, padlen), np.float32)
    padG[:, :, H:H + NV] = pred

    # global const rows [CW, NV] (+1 bias row appended per core below)
    CG = np.zeros((CW, NV), np.float32)
    WS = Wk.sum(axis=0) * np.float32(STAB)   # (N,3)
    for k in range(K):
        for j in range(3):
            CG[k * 3 + j] = Wk[k, :, j] * np.float32(STAB)
    for j in range(3):
        CG[3 * K + j] = WS[:, j]
    for k in range(K):
        for i in range(3):
            CG[3 * K + 3 + k * 3 + i] = tk[k, :, i]
    for k in range(K):
        CG[6 * K + 3 + k] = wk[k]

    in_maps = []
    for c in range(NCORES):
        base = c * NC_V
        # grouped pred windows: (B, 3, G, P, GWD); partition p covers
        # vertices [base + p*FQ, base + p*FQ + FQ), window g starts at
        # offset bases[g] - so slot s within the window is offset
        # bases[g] + s.
        wins = np.empty((B, 3, G, P, GWD), np.float32)
        pidx = (np.arange(P)[:, None] * FQ + np.arange(GWD)[None, :])  # (P,GWD)
        for g, bg in enumerate(bases):
            idx = H + base + bg + pidx                                 # (P,GWD)
            wins[:, :, g, :, :] = padG[:, :, idx]
        predl = np.ascontiguousarray(
            wins.transpose(3, 0, 1, 2, 4)
        ).reshape(P, B * 3 * G * GWD)

        cc = np.zeros((CW + 1, VP), np.float32)
        hi = min(base + VP, NV) - base
        hi = min(hi, NC_V)                   # zero weights on padded tail
        cc[:CW, :hi] = CG[:, base:base + hi]
        cc[CW, :] = C_SINL                   # activation bias row (2pi/3)
        constl = np.ascontiguousarray(
            cc.reshape(CW + 1, P, FQ).transpose(1, 0, 2)
        ).reshape(P, (CW + 1) * FQ)

        in_maps.append({"predl": predl, "constl": constl})
    return in_maps, (G, GWD, tuple(win_map)), CW


# ---------------------------------------------------------------------------
# Device kernel builder
# ---------------------------------------------------------------------------

def _build_nc(K, wingeo):
    G, GWD, win_map = wingeo
    CW = 7 * K + 3
    FD = BQ * FQ

    nc = bacc.Bacc("TRN2", target_bir_lowering=False, debug=False,
                   num_devices=NCORES)

    predl_d = nc.dram_tensor("predl", [P, B * 3 * G * GWD], F32,
                             kind="ExternalInput").ap()
    constl_d = nc.dram_tensor("constl", [P, (CW + 1) * FQ], F32,
                              kind="ExternalInput").ap()
    out_d = nc.dram_tensor("out", [P, B], F32, kind="ExternalOutput").ap()

    with tile.TileContext(nc) as tc, ExitStack() as ctx:
        cpool = ctx.enter_context(tc.tile_pool(name="consts", bufs=1))
        ppool = ctx.enter_context(tc.tile_pool(name="pred", bufs=2))
        wpool = ctx.enter_context(tc.tile_pool(name="work", bufs=72))
        opool = ctx.enter_context(tc.tile_pool(name="outp", bufs=1))

        consts = cpool.tile([P, (CW + 1) * FQ], F32)
        nc.sync.dma_start(consts[:, :], constl_d[:, :])
        bias_sinl = consts[:, CW * FQ:CW * FQ + 1]   # [128,1] holding 2pi/3

        outacc = opool.tile([P, B], F32)

        def cview(qi):
            """Const row qi broadcast over BQ: [P, BQ, FQ] stride-0 AP."""
            a = consts[:, qi * FQ:(qi + 1) * FQ]
            return bass.AP(a.tensor, a.offset,
                           [list(a.ap[0]), [0, BQ], list(a.ap[1])])

        c_Wk = lambda k, j: cview(k * 3 + j)
        c_WS = lambda j: cview(3 * K + j)
        c_tk = lambda k, i: cview(3 * K + 3 + k * 3 + i)
        c_wk = lambda k: cview(6 * K + 3 + k)

        vec = nc.vector
        act = nc.scalar

        # bf16 copy of the tk/wk const rows (rows 3K+3 .. 7K+3, contiguous)
        BFc = mybir.dt.bfloat16
        cbf = cpool.tile([P, 4 * K * FQ], BFc)
        vec.tensor_copy(cbf[:, :],
                        consts[:, (3 * K + 3) * FQ:(7 * K + 3) * FQ])

        def cviewb(qi):
            a = cbf[:, qi * FQ:(qi + 1) * FQ]
            return bass.AP(a.tensor, a.offset,
                           [list(a.ap[0]), [0, BQ], list(a.ap[1])])

        c_tkb = lambda k, i: cviewb(k * 3 + i)
        c_wkb = lambda k: cviewb(3 * K + k)

        def quarter(qb):
            pq = ppool.tile([P, BQ * 3 * G * GWD], F32, tag="pq")
            span = BQ * 3 * G * GWD
            nc.sync.dma_start(pq[:, :], predl_d[:, qb * span:(qb + 1) * span])

            def qv(i, w):
                """Shifted-window view [P, BQ, FQ] of pq: component i,
                window index w (0=center, 1..K=offset classes)."""
                g, slot = win_map[w]
                base = (i * G + g) * GWD + slot
                a = pq[:, :]
                return bass.AP(a.tensor, a.offset + base,
                               [list(a.ap[0]), [3 * G * GWD, BQ], [1, FQ]])

            def wt(name, dt=F32):
                tag = "work" if dt == F32 else "workb"
                nbufs = 22 if dt == F32 else 48
                t = wpool.tile([P, FD], dt, tag=tag, name=name,
                               uniquify=True, bufs=nbufs)
                a = t[:, :]
                return bass.AP(a.tensor, a.offset,
                               [list(a.ap[0]), [FQ, BQ], [1, FQ]])

            def wtp(name, nent, dt, tag, nbufs):
                """packed tile [P, nent*FD]; returns raw AP."""
                return wpool.tile([P, nent * FD], dt, tag=tag, name=name,
                                  uniquify=True, bufs=nbufs)[:, :]

            def pent(t, ent):
                """single-entry view [P, BQ, FQ] of a packed tile."""
                return bass.AP(t.tensor, t.offset + ent * FD,
                               [list(t.ap[0]), [FQ, BQ], [1, FQ]])

            def ptri(t, off, estride=FD):
                """3-entry view [P, 3, BQ, FQ] starting at element offset."""
                return bass.AP(t.tensor, t.offset + off,
                               [list(t.ap[0]), [estride, 3], [FQ, BQ],
                                [1, FQ]])

            def bview3(a3):
                """broadcast a [P, BQ, FQ] AP to [P, 3, BQ, FQ]."""
                return bass.AP(a3.tensor, a3.offset,
                               [list(a3.ap[0]), [0, 3]] +
                               [list(d) for d in a3.ap[1:]])

            BF = mybir.dt.bfloat16

            def cast(src, name):
                dst = wt(name, BF)
                act.copy(dst, src)        # casts ride the idle ACT engine
                return dst

            gps = nc.gpsimd

            def tt(op, out, a, b, eng=None):
                (eng or vec).tensor_tensor(out=out, in0=a, in1=b, op=op)

            def mac_list(out, terms, tmp, eng=None):
                """out = sum of products; terms = [(a, b), ...]."""
                (a0, b0) = terms[0]
                tt(AL.mult, out, a0, b0, eng)
                for (a, b) in terms[1:]:
                    tt(AL.mult, tmp, a, b, eng)
                    tt(AL.add, out, out, tmp, eng)

            tmp = wt("tmp")
            tmp2 = wt("tmp2")
            tmpb = wt("tmpb", BF)
            tmpb2 = wt("tmpb2", BF)

            # ---- A = stab * (sum_k q_k Wk^T - p WS^T), packed (i,j) ----
            # One instruction computes all three j-columns of row i:
            # out[j-triple] = q(i,k) [bcast j] * Wk[k, j-triple].
            def c_row3(row0):
                a = consts[:, row0 * FQ:(row0 + 3) * FQ]
                return bass.AP(a.tensor, a.offset,
                               [list(a.ap[0]), [FQ, 3], [0, BQ], [1, FQ]])

            ApAll = wtp("ApAll", 9, F32, "pkA", 2)
            tmp3 = wtp("tmp3", 3, F32, "pk3f", 1)
            t3v = ptri(tmp3, 0)
            A = [[pent(ApAll, i * 3 + j) for j in range(3)] for i in range(3)]
            for i in range(3):
                dst = ptri(ApAll, i * 3 * FD)
                vec.tensor_tensor(out=dst, in0=bview3(qv(i, 1)),
                                  in1=c_row3(0), op=AL.mult)
                for k in range(1, K):
                    vec.tensor_tensor(out=t3v, in0=bview3(qv(i, k + 1)),
                                      in1=c_row3(k * 3), op=AL.mult)
                    vec.tensor_tensor(out=dst, in0=dst, in1=t3v, op=AL.add)
                vec.tensor_tensor(out=t3v, in0=bview3(qv(i, 0)),
                                  in1=c_row3(3 * K), op=AL.mult)
                vec.tensor_tensor(out=dst, in0=dst, in1=t3v, op=AL.subtract)

            # ---- cast packed A to bf16 early (feeds AV, T2, Z/AW) ----
            Abp = wtp("Abp", 9, BF, "pkAb", 2)
            act.copy(Abp, ApAll)
            Ab = [[pent(Abp, i * 3 + j) for j in range(3)] for i in range(3)]

            # ---- AV = A^T A in bf16; diagonal via ACT squares ----
            av = {}
            for a in range(3):
                v = wt(f"av{a}{a}", BF)
                s1t, s2t, s3t = wt("avs1", BF), wt("avs2", BF), wt("avs3", BF)
                act.square(s1t, Ab[0][a])
                act.square(s2t, Ab[1][a])
                act.square(s3t, Ab[2][a])
                tt(AL.add, v, s1t, s2t)
                tt(AL.add, v, v, s3t)
                av[(a, a)] = v
            for (a, b) in ((0, 1), (0, 2), (1, 2)):
                v = wt(f"av{a}{b}", BF)
                mac_list(v, [(Ab[i][a], Ab[i][b]) for i in range(3)], tmpb)
                av[(a, b)] = v
            av00, av01, av02 = av[(0, 0)], av[(0, 1)], av[(0, 2)]
            av11, av12, av22 = av[(1, 1)], av[(1, 2)], av[(2, 2)]

            # ---- detA and its sign ----
            detA = wt("detA")
            u0, u1, u2 = wt("u0"), wt("u1"), wt("u2")
            tt(AL.mult, u0, A[1][1], A[2][2])
            tt(AL.mult, tmp, A[2][1], A[1][2])
            tt(AL.subtract, u0, u0, tmp)
            tt(AL.mult, u1, A[0][1], A[2][2])
            tt(AL.mult, tmp, A[2][1], A[0][2])
            tt(AL.subtract, u1, u1, tmp)
            tt(AL.mult, u2, A[0][1], A[1][2])
            tt(AL.mult, tmp, A[1][1], A[0][2])
            tt(AL.subtract, u2, u2, tmp)
            tt(AL.mult, detA, A[0][0], u0)
            tt(AL.mult, tmp, A[1][0], u1)
            tt(AL.subtract, detA, detA, tmp)
            tt(AL.mult, tmp, A[2][0], u2)
            tt(AL.add, detA, detA, tmp)
            dsg = wt("dsg")
            act.activation(dsg, detA, AF.Sign)

            yield   # head/tail split for software-pipelined emission

            # ---- trig eigenvalues (bf16 polynomial part; f32 acos chain) ----
            sqb01, sqb02, sqb12 = wt("sqb01", BF), wt("sqb02", BF), wt("sqb12", BF)
            act.square(sqb01, av01)
            act.square(sqb02, av02)
            act.square(sqb12, av12)
            p1 = wt("p1", BF)
            tt(AL.add, p1, sqb01, sqb02)
            tt(AL.add, p1, p1, sqb12)
            trb = wt("trb", BF)
            tt(AL.add, trb, av00, av11)
            tt(AL.add, trb, trb, av22)
            qm = wt("qm", BF)
            act.mul(qm, trb, 1.0 / 3.0)
            b00, b11, b22 = wt("b00", BF), wt("b11", BF), wt("b22", BF)
            tt(AL.subtract, b00, av00, qm)
            tt(AL.subtract, b11, av11, qm)
            tt(AL.subtract, b22, av22, qm)
            sq1, sq2, sq3 = wt("sq1", BF), wt("sq2", BF), wt("sq3", BF)
            act.square(sq1, b00)
            act.square(sq2, b11)
            act.square(sq3, b22)
            p2 = wt("p2", BF)
            tt(AL.add, p2, sq1, sq2)
            tt(AL.add, p2, p2, sq3)
            # p2 = p2 + 2*p1 ; clamp
            vec.scalar_tensor_tensor(out=p2, in0=p1, scalar=2.0, in1=p2,
                                     op0=AL.mult, op1=AL.add)
            vec.tensor_scalar_max(out=p2, in0=p2, scalar1=1e-18)
            # ln((2p)^2) = ln(p2 * 4/6); exp(0.5*..) = 2p; exp(-1.5*..) = 1/(8p^3)
            lnp6 = wt("lnp6")
            act.activation(lnp6, p2, AF.Ln, scale=4.0 / 6.0)
            two_p = wt("two_p")
            act.activation(two_p, lnp6, AF.Exp, scale=0.5)
            pinv8 = wt("pinv8")
            act.activation(pinv8, lnp6, AF.Exp, scale=-1.5)
            # detC with diagonal b00/b11/b22, off-diag av01/av02/av12 (bf16)
            detC = wt("detC", BF)
            ub0, ub1, ub2 = wt("ub0", BF), wt("ub1", BF), wt("ub2", BF)
            tt(AL.mult, ub0, b11, b22)
            tt(AL.subtract, ub0, ub0, sqb12)
            tt(AL.mult, ub1, av01, b22)
            tt(AL.mult, tmpb, av12, av02)
            tt(AL.subtract, ub1, ub1, tmpb)
            tt(AL.mult, ub2, av01, av12)
            tt(AL.mult, tmpb, b11, av02)
            tt(AL.subtract, ub2, ub2, tmpb)
            tt(AL.mult, detC, b00, ub0)
            tt(AL.mult, tmpb, av01, ub1)
            tt(AL.subtract, detC, detC, tmpb)
            tt(AL.mult, tmpb, av02, ub2)
            tt(AL.add, detC, detC, tmpb)
            # r = detC / (2 p^3) = (detC * 4) * pinv8   (f32 chain)
            r = wt("r")
            vec.scalar_tensor_tensor(out=r, in0=detC, scalar=4.0, in1=pinv8,
                                     op0=AL.mult, op1=AL.mult)
            vec.tensor_scalar(out=r, in0=r, scalar1=RCLAMP, scalar2=-RCLAMP,
                              op0=AL.min, op1=AL.max)
            r2 = wt("r2")
            act.square(r2, r)
            lnomr = wt("lnomr")
            act.activation(lnomr, r2, AF.Ln, bias=1.0, scale=-1.0)
            eh = wt("eh")
            act.activation(eh, lnomr, AF.Exp, scale=-0.5)
            s_ = wt("s_")
            tt(AL.mult, s_, r, eh)
            at = wt("at")
            act.activation(at, s_, AF.Arctan)
            sinL, sinM = wt("sinL", BF), wt("sinM", BF)
            act.activation(sinL, at, AF.Sin, bias=bias_sinl, scale=-1.0 / 3.0)
            act.activation(sinM, at, AF.Sin, scale=-1.0 / 3.0)
            two_pb = cast(two_p, "two_pb")
            lam3, lam2, lam1 = wt("lam3", BF), wt("lam2", BF), wt("lam1", BF)
            tt(AL.mult, tmpb, two_pb, sinL)
            tt(AL.add, lam3, qm, tmpb)
            tt(AL.mult, tmpb, two_pb, sinM)
            tt(AL.add, lam2, qm, tmpb)
            tt(AL.subtract, tmpb, trb, lam3)
            tt(AL.subtract, lam1, tmpb, lam2)
            d32 = wt("d32", BF)
            tt(AL.subtract, tmpb, sinL, sinM)
            tt(AL.mult, d32, two_pb, tmpb)
            d21, d31 = wt("d21", BF), wt("d31", BF)
            tt(AL.subtract, d21, lam2, lam1)
            tt(AL.subtract, d31, lam3, lam1)
            l2c, l3c = wt("l2c", BF), wt("l3c", BF)
            vec.tensor_scalar_max(out=l2c, in0=lam2, scalar1=CLIPV)
            vec.tensor_scalar_max(out=l3c, in0=lam3, scalar1=CLIPV)
            g2, g3 = wt("g2", BF), wt("g3", BF)
            act.activation(tmp, l2c, AF.Ln)
            act.activation(g2, tmp, AF.Exp, scale=-0.5)
            act.activation(tmp, l3c, AF.Ln)
            act.activation(g3, tmp, AF.Exp, scale=-0.5)
            l3sq = wt("l3sq", BF)
            act.square(l3sq, l3c)

            def safe_recip(dst, x, tmpa, tmpf):
                """dst = sign(x)/max(|x|, 1e-6*l3sq); bf16 except Ln stage."""
                act.activation(tmpa, x, AF.Abs)
                vec.scalar_tensor_tensor(out=tmpa, in0=l3sq, scalar=1e-6,
                                         in1=tmpa, op0=AL.mult, op1=AL.max)
                act.activation(tmpf, tmpa, AF.Ln)
                act.activation(dst, tmpf, AF.Exp, scale=-1.0)
                act.activation(tmpa, x, AF.Sign)
                tt(AL.mult, dst, dst, tmpa)

            den2m, den3 = wt("den2m", BF), wt("den3", BF)
            tt(AL.mult, den2m, d21, d32)
            tt(AL.mult, den3, d31, d32)
            inv2m, inv3 = wt("inv2m", BF), wt("inv3", BF)
            safe_recip(inv2m, den2m, tmpb2, tmp)
            safe_recip(inv3, den3, tmpb2, tmp)
            gam2b, gam3b = wt("gam2b", BF), wt("gam3b", BF)
            # gam2 = -g2*inv2m  (den2 = -den2m)
            vec.scalar_tensor_tensor(out=gam2b, in0=g2, scalar=-1.0,
                                     in1=inv2m, op0=AL.mult, op1=AL.mult)
            tt(AL.mult, gam3b, g3, inv3)

            # ---- T2 = (AV - l1)(AV - l3), T3 = T2 + d32*(AV - l1) ----
            n00b, n11b, n22b = wt("n00b", BF), wt("n11b", BF), wt("n22b", BF)
            m00b, m11b, m22b = wt("m00b", BF), wt("m11b", BF), wt("m22b", BF)
            tt(AL.subtract, n00b, av00, lam1)
            tt(AL.subtract, n11b, av11, lam1)
            tt(AL.subtract, n22b, av22, lam1)
            tt(AL.subtract, m00b, av00, lam3)
            tt(AL.subtract, m11b, av11, lam3)
            tt(AL.subtract, m22b, av22, lam3)
            a01b, a02b, a12b = av01, av02, av12
            d32b = d32
            dsgb = cast(dsg, "dsgb")
            g3db = wt("g3db", BF)
            tt(AL.mult, g3db, gam3b, dsgb)

            sym_idx = ("00", "01", "02", "11", "12", "22")
            T2 = {s: wt(f"T2{s}", BF) for s in sym_idx}
            # diagonal entries: one mult + two adds each
            tt(AL.mult, T2["00"], n00b, m00b)
            tt(AL.add, T2["00"], T2["00"], sqb01)
            tt(AL.add, T2["00"], T2["00"], sqb02)
            tt(AL.mult, T2["11"], n11b, m11b)
            tt(AL.add, T2["11"], T2["11"], sqb01)
            tt(AL.add, T2["11"], T2["11"], sqb12)
            tt(AL.mult, T2["22"], n22b, m22b)
            tt(AL.add, T2["22"], T2["22"], sqb02)
            tt(AL.add, T2["22"], T2["22"], sqb12)
            t2_terms = {
                "01": [(n00b, a01b), (a01b, m11b), (a02b, a12b)],
                "02": [(n00b, a02b), (a01b, a12b), (a02b, m22b)],
                "12": [(a01b, a02b), (n11b, a12b), (a12b, m22b)],
            }
            for s in ("01", "02", "12"):
                mac_list(T2[s], t2_terms[s], tmpb)
            N1 = {"00": n00b, "11": n11b, "22": n22b,
                  "01": a01b, "02": a02b, "12": a12b}
            # Zs = gam2*T2 + gam3*T3 ; W2 = gam2*T2 + dsg*gam3*T3
            Zs = {s: wt(f"Zs{s}", BF) for s in sym_idx}
            W2 = {s: wt(f"W2{s}", BF) for s in sym_idx}
            for s in sym_idx:
                t3 = wt(f"T3{s}", BF)
                tt(AL.mult, tmpb, d32b, N1[s])
                tt(AL.add, t3, T2[s], tmpb)
                tt(AL.mult, tmpb, gam2b, T2[s])     # gam2*T2
                tt(AL.mult, tmpb2, gam3b, t3)
                tt(AL.add, Zs[s], tmpb, tmpb2)
                tt(AL.mult, tmpb2, g3db, t3)
                tt(AL.add, W2[s], tmpb, tmpb2)

            # ---- Z = A @ Zs ; AW2 = A @ W2 (3x3 @ sym) ----
            def sym_get(S, a, b):
                return S["".join(map(str, sorted((a, b))))]

            # i-packed 3x3 @ sym products (Abp cast earlier)
            Zp = wtp("Zp", 9, BF, "pkZ", 4)
            AWp = wtp("AWp", 9, BF, "pkZ", 4)
            tmpb3 = wtp("tmpb3", 3, BF, "pk3b", 3)
            tb3 = ptri(tmpb3, 0)
            for (S, dstp) in ((Zs, Zp), (W2, AWp)):
                for j in range(3):
                    # out[i-triple at column j] = sum_kk A[i][kk]*S(kk,j)
                    dst = ptri(dstp, j * FD, estride=3 * FD)
                    vec.tensor_tensor(out=dst,
                                      in0=ptri(Abp, 0, estride=3 * FD),
                                      in1=bview3(sym_get(S, 0, j)),
                                      op=AL.mult)
                    for kk in (1, 2):
                        vec.tensor_tensor(out=tb3,
                                          in0=ptri(Abp, kk * FD,
                                                   estride=3 * FD),
                                          in1=bview3(sym_get(S, kk, j)),
                                          op=AL.mult)
                        vec.tensor_tensor(out=dst, in0=dst, in1=tb3,
                                          op=AL.add)
            Z = [[pent(Zp, i * 3 + j) for j in range(3)] for i in range(3)]
            AW = [[pent(AWp, i * 3 + j) for j in range(3)] for i in range(3)]

            # ---- R = AW + dsg * cof(Z) ----
            cof_pairs = {
                (0, 0): ((1, 1), (2, 2), (1, 2), (2, 1)),
                (0, 1): ((1, 2), (2, 0), (1, 0), (2, 2)),
                (0, 2): ((1, 0), (2, 1), (1, 1), (2, 0)),
                (1, 0): ((2, 1), (0, 2), (2, 2), (0, 1)),
                (1, 1): ((2, 2), (0, 0), (2, 0), (0, 2)),
                (1, 2): ((2, 0), (0, 1), (2, 1), (0, 0)),
                (2, 0): ((0, 1), (1, 2), (0, 2), (1, 1)),
                (2, 1): ((0, 2), (1, 0), (0, 0), (1, 2)),
                (2, 2): ((0, 0), (1, 1), (0, 1), (1, 0)),
            }
            Rp = wtp("Rp", 9, BF, "pkZ", 4)
            R = [[pent(Rp, i * 3 + j) for j in range(3)] for i in range(3)]
            for i in range(3):
                for j in range(3):
                    (pa, pb, pc, pd) = cof_pairs[(i, j)]
                    cf = wt(f"cf{i}{j}", BF)
                    tt(AL.mult, cf, Z[pa[0]][pa[1]], Z[pb[0]][pb[1]])
                    tt(AL.mult, tmpb, Z[pc[0]][pc[1]], Z[pd[0]][pd[1]])
                    tt(AL.subtract, cf, cf, tmpb)
                    tt(AL.mult, cf, cf, dsgb)
                    tt(AL.add, R[i][j], AW[i][j], cf)

            # ---- energy (bf16 residual chain; i-packed rte from Rp) ----
            nrg = wt("nrg", BF)
            rteP = wtp("rteP", 3, BF, "pk3b", 3)
            rte = [pent(rteP, i) for i in range(3)]
            dpb = wt("dpb", BF)
            dfc, ns = wt("dfc", BF), wt("ns", BF)
            sqd = wt("sqd", BF)
            rtv = ptri(rteP, 0)
            for k in range(K):
                # rte[i-triple] = sum_j R[i][j] * t_k[j]
                vec.tensor_tensor(out=rtv,
                                  in0=ptri(Rp, 0, estride=3 * FD),
                                  in1=bview3(c_tkb(k, 0)), op=AL.mult)
                for j in (1, 2):
                    vec.tensor_tensor(out=tb3,
                                      in0=ptri(Rp, j * FD, estride=3 * FD),
                                      in1=bview3(c_tkb(k, j)), op=AL.mult)
                    vec.tensor_tensor(out=rtv, in0=rtv, in1=tb3, op=AL.add)
                for i in range(3):
                    # dp = q - p (f32 sub, bf16 out); diff = dp - rte
                    tt(AL.subtract, dpb, qv(i, k + 1), qv(i, 0))
                    tt(AL.subtract, dfc, dpb, rte[i])
                    if i == 0:
                        act.square(ns, dfc)
                    else:
                        act.square(sqd, dfc)
                        tt(AL.add, ns, ns, sqd)
                nrm = wt("nrm", BF)
                act.activation(nrm, ns, AF.Sqrt)
                if k == 0:
                    tt(AL.mult, nrg, nrm, c_wkb(k))
                else:
                    tt(AL.mult, tmpb, nrm, c_wkb(k))
                    tt(AL.add, nrg, nrg, tmpb)
            vec.tensor_scalar_min(out=nrg, in0=nrg, scalar1=1.0)
            vec.tensor_reduce(out=outacc[:, qb * BQ:(qb + 1) * BQ],
                              in_=nrg, axis=mybir.AxisListType.X, op=AL.add)

        gens = [quarter(qb) for qb in range(NQ)]
        next(gens[0])
        for qb in range(1, NQ):
            next(gens[qb])
            for _ in gens[qb - 1]:
                pass
        for _ in gens[NQ - 1]:
            pass

        nc.sync.dma_start(out_d[:, :], outacc[:, :])

    nc.compile()          # bacc register allocation / DCE / nop fusion
    return nc


def _get_nc(K, wingeo):
    key = (K, wingeo)
    if key not in _nc_cache:
        _nc_cache[key] = _build_nc(K, wingeo)
    return _nc_cache[key]


# ---------------------------------------------------------------------------
# Entry point
# ---------------------------------------------------------------------------

def _install_ntff_shim():
    """Provide antenv.axon_hooks (missing in this image) so
    run_bass_kernel_spmd(trace=True) can reach the NTFF profiler in
    libaxon_pjrt.so."""
    import types

    try:
        import antenv.axon_hooks  # noqa: F401
        return True
    except ImportError:
        pass
    try:
        import antenv
        from trn_agent_boot.trn_boot import _ntff_profile_via_ctypes
    except ImportError:
        return False
    mod = types.ModuleType("antenv.axon_hooks")
    state = {"hook": None}
    mod.set_axon_ntff_profile_hook = lambda h: state.__setitem__("hook", h)
    mod.get_axon_ntff_profile_hook = lambda: state["hook"]
    sys.modules["antenv.axon_hooks"] = mod
    antenv.axon_hooks = mod
    try:
        hook = _ntff_profile_via_ctypes("/opt/axon/libaxon_pjrt.so")
    except OSError:
        hook = None
    if hook is not None:
        mod.set_axon_ntff_profile_hook(hook)
    return hook is not None


def kernel(**inputs) -> np.ndarray:
    pred = np.asarray(inputs["prediction"], np.float32)
    adj_idx = np.asarray(inputs["adj_list_indices"])
    adj_w = np.asarray(inputs["adj_list_weights"], np.float32)
    tev_T = np.asarray(inputs["template_edge_vectors_T"], np.float32)
    tev_w = np.asarray(inputs["template_ev_weighted"], np.float32)

    offs, wk, Wk, tk = _build_offset_classes(adj_idx, adj_w, tev_T, tev_w)
    K = len(offs)
    in_maps, wingeo, CW = _host_prepare(pred, offs, wk, Wk, tk)

    nc = _get_nc(K, wingeo)
    import os
    trace = bool(int(os.environ.get("ARAP_TRACE", "0")))
    if trace:
        trace = _install_ntff_shim()
    try:
        res = run_bass_kernel_spmd(nc, in_maps, core_ids=list(range(NCORES)),
                                   trace=trace)
    except Exception:
        if not trace:
            raise
        res = run_bass_kernel_spmd(nc, in_maps, core_ids=list(range(NCORES)),
                                   trace=False)
    kernel._last_exec_ns = res.exec_time_ns
    kernel._last_results = res

    total = np.zeros(B, np.float64)
    for c in range(NCORES):
        total += res.results[c]["out"].astype(np.float64).sum(axis=0)
    return (total / NV).astype(np.float32)


kernel._last_exec_ns = None



# revision 6
# speedup vs baseline: 1.5786x; 1.5786x over previous
"""ARAP loss kernel for Trainium2 (8 NeuronCores, SPMD over the vertex axis).

Problem: nn_ArapLoss — per-vertex 6-neighbor gather on a 316x316 grid mesh,
3x3 polar decomposition (closed-form symmetric eigenanalysis) per vertex,
cotan-weighted edge-residual energy, clamped mean over vertices.

Strategy (v2 — vector-engine lean)
----------------------------------
- Shard the vertex axis N=99856 across 8 cores (12482 each, padded to
  12544 = 128*98).  Grid adjacency reduces to K=6 constant index offsets
  {+-1, +-316, +-317}; the host materializes shifted windows of
  `prediction` so the device does no gather.
- Edge vectors e_k = q_{n+o_k} - p_n are computed ONCE in f32 and stored
  bf16; everything downstream runs in bf16 (DVE 2x mode).
- The template-edge xy components are EXACTLY {0,+-1} per offset class
  (regular grid), so A = sum_k e_k (stab w_k t_k)^T collapses to signed
  sums plus one weighted z-column, and the rotated-template residual
  e_k - R t_k collapses to (e_k - tz_k R[:,2]) -+ R-column combos.
- R from a SINGLE 3x3 product:  R = Y + cof(Y),  Y = A (g2 P2 + d g3 P3).
  cof(u2 v2' + d u3 v3') = d^2 u1 v1' = u1 v1', so the smallest-eigenvalue
  component needs no division by s1 and no second product / sign fixup.
- Output: per-core partial sums [128, B]; host reduces and divides by N.
"""
import sys

for _p in ("/opt/trn_rl_repo", "/opt/trn_rl_repo/concourse", "/opt/pypackages"):
    if _p not in sys.path:
        sys.path.insert(0, _p)

from contextlib import ExitStack

import ml_dtypes
import numpy as np

import concourse.bass as bass
import concourse.tile as tile
from concourse import bacc, mybir
from concourse.bass_utils import run_bass_kernel_spmd

F32 = mybir.dt.float32
BF = mybir.dt.bfloat16
AL = mybir.AluOpType
AF = mybir.ActivationFunctionType

# ---- problem geometry (hardcoded per spec) --------------------------------
B = 16
NV = 99856
NCORES = 8
P = 128
NC_V = NV // NCORES            # 12482 real vertices per core
FQ = 98                        # free-dim vertices per partition
VP = P * FQ                    # 12544 padded vertices per core
BQ = 4                         # batch elements per pass
NQ = B // BQ
K = 6
STAB = 1000.0
CLIPV = 1e-6                   # 1e-12 * stab^2
C_SINL = float(2.0 * np.pi / 3.0)
RCLAMP = 1.0 - 1e-6
OFFS = (-317, -316, -1, 1, 316, 317)
# xy components of template edges per offset class (exact on the grid)
CX = (-1, -1, 0, 0, 1, 1)
CY = (-1, 0, -1, 1, 0, 1)

_nc_cache = {}


# ---------------------------------------------------------------------------
# Host-side preprocessing
# ---------------------------------------------------------------------------

def _build_offset_classes(adj_idx, adj_w, tev_T):
    """(N,D) adjacency -> per-offset-class weights wk (K,N) and template
    edge z-components tzk (K,N).  Asserts the grid structure this kernel
    hardcodes (xy components == CX/CY per class)."""
    N, D = adj_idx.shape
    ar = np.arange(N, dtype=np.int64)
    real = (adj_idx > 0) | (np.arange(D)[None, :] == 0)
    delta = np.asarray(adj_idx, np.int64) - ar[:, None]
    offs = np.unique(delta[real])
    assert tuple(int(o) for o in offs) == OFFS, f"unexpected offsets {offs}"
    wk = np.zeros((K, N), np.float32)
    tzk = np.zeros((K, N), np.float32)
    for k, o in enumerate(OFFS):
        sel = real & (delta == o)
        n_id, d_id = np.nonzero(sel)
        wk[k, n_id] = adj_w[n_id, d_id]
        tzk[k, n_id] = tev_T[n_id, 2, d_id]
        has = wk[k] != 0
        assert np.all(tev_T[has, 0, :][sel[has]] == CX[k])
        assert np.all(tev_T[has, 1, :][sel[has]] == CY[k])
    return wk, tzk


def _group_offsets(gap=8):
    """Group [0]+OFFS into consecutive runs; returns (bases, width, win_map)
    where win_map[x] = (g, slot) for x in [0(center)] + OFFS order."""
    allo = sorted(set([0] + list(OFFS)))
    groups = [[allo[0]]]
    for o in allo[1:]:
        if o - groups[-1][-1] <= gap:
            groups[-1].append(o)
        else:
            groups.append([o])
    bases = [g[0] for g in groups]
    width = FQ + max(g[-1] - g[0] for g in groups) + 1
    lut = {}
    for gi, g in enumerate(groups):
        for o in g:
            lut[o] = (gi, o - g[0])
    win_map = [lut[0]] + [lut[o] for o in OFFS]
    return bases, width, tuple(win_map)


def _host_prepare(pred, wk, tzk):
    """Build per-core input maps: predl [P, B*3*G*GWD] f32 and
    constb [P, 24*FQ] bf16 (rows: wp(6), wz(6), tz(6), wk(6))."""
    bases, GWD, win_map = _group_offsets()
    G = len(bases)
    H = max(max(abs(o) for o in OFFS), 1)
    padlen = NV + 2 * H + (VP - NC_V) + GWD
    padG = np.zeros((B, 3, padlen), np.float32)
    padG[:, :, H:H + NV] = pred

    wp = wk * np.float32(STAB)           # (K, N)
    wz = wp * tzk
    CG = np.concatenate([wp, wz, tzk, wk], axis=0)   # (24, N)

    in_maps = []
    pidx = (np.arange(P)[:, None] * FQ + np.arange(GWD)[None, :])  # (P,GWD)
    for c in range(NCORES):
        base = c * NC_V
        wins = np.empty((B, 3, G, P, GWD), np.float32)
        for g, bg in enumerate(bases):
            idx = H + base + bg + pidx
            wins[:, :, g, :, :] = padG[:, :, idx]
        predl = np.ascontiguousarray(
            wins.transpose(3, 0, 1, 2, 4)
        ).reshape(P, B * 3 * G * GWD)

        cc = np.zeros((24, VP), np.float32)
        hi = min(base + VP, NV) - base
        hi = min(hi, NC_V)                   # zero weights on padded tail
        cc[:, :hi] = CG[:, base:base + hi]
        constb = np.ascontiguousarray(
            cc.reshape(24, P, FQ).transpose(1, 0, 2)
        ).reshape(P, 24 * FQ).astype(ml_dtypes.bfloat16)

        in_maps.append({"predl": predl, "constb": constb})
    return in_maps, (G, GWD, win_map)


# ---------------------------------------------------------------------------
# Device kernel builder
# ---------------------------------------------------------------------------

def _build_nc(wingeo):
    G, GWD, win_map = wingeo
    FD = BQ * FQ

    nc = bacc.Bacc("TRN2", target_bir_lowering=False, debug=False,
                   num_devices=NCORES)

    predl_d = nc.dram_tensor("predl", [P, B * 3 * G * GWD], F32,
                             kind="ExternalInput").ap()
    constb_d = nc.dram_tensor("constb", [P, 24 * FQ], BF,
                              kind="ExternalInput").ap()
    out_d = nc.dram_tensor("out", [P, B], F32, kind="ExternalOutput").ap()

    vec = None
    act = None

    with tile.TileContext(nc) as tc, ExitStack() as ctx:
        cpool = ctx.enter_context(tc.tile_pool(name="consts", bufs=1))
        ppool = ctx.enter_context(tc.tile_pool(name="pred", bufs=2))
        wpool = ctx.enter_context(tc.tile_pool(name="work", bufs=80))

        cb = cpool.tile([P, 24 * FQ], BF)
        nc.sync.dma_start(cb[:, :], constb_d[:, :])
        outacc = cpool.tile([P, B], F32)
        bias_sinl = cpool.tile([P, 1], F32)
        nc.gpsimd.memset(bias_sinl[:, :], C_SINL)

        vec = nc.vector
        act = nc.scalar

        def crow(r):
            """bf16 const row r as [P, BQ, FQ] (batch-broadcast)."""
            a = cb[:, r * FQ:(r + 1) * FQ]
            return bass.AP(a.tensor, a.offset,
                           [list(a.ap[0]), [0, BQ], list(a.ap[1])])

        def crow3(r):
            """bf16 const row r as [P, 3, BQ, FQ] (i- and batch-bcast)."""
            a = cb[:, r * FQ:(r + 1) * FQ]
            return bass.AP(a.tensor, a.offset,
                           [list(a.ap[0]), [0, 3], [0, BQ], list(a.ap[1])])

        def wrow6():
            """wk rows 18..23 as [P, 6, BQ, FQ]."""
            a = cb[:, 18 * FQ:24 * FQ]
            return bass.AP(a.tensor, a.offset,
                           [list(a.ap[0]), [FQ, 6], [0, BQ], [1, FQ]])

        r_wp = lambda k: crow3(k)
        r_wz = lambda k: crow3(6 + k)
        r_tz = lambda k: crow3(12 + k)

        def tt(op, out, a, b):
            vec.tensor_tensor(out=out, in0=a, in1=b, op=op)

        def pass_gen(qb):
            pq = ppool.tile([P, BQ * 3 * G * GWD], F32, tag="pq")
            span = BQ * 3 * G * GWD
            nc.sync.dma_start(pq[:, :], predl_d[:, qb * span:(qb + 1) * span])

            def wt(name, dt=BF, n=1, tag=None, bufs=None):
                """work tile [P, n*FD]"""
                if tag is None:
                    tag = {(BF, 1): "sg", (F32, 1): "sf",
                           (BF, 3): "t3", (BF, 6): "s6", (BF, 9): "pk9"}[
                               (dt, n)]
                if bufs is None:
                    bufs = {"sg": 34, "sf": 10, "t3": 10, "s6": 5,
                            "pk9": 3}[tag]
                return wpool.tile([P, n * FD], dt, tag=tag, name=name,
                                  uniquify=True, bufs=bufs)

            def ent(t, s=0):
                """single entry view [P, BQ, FQ] at slot s."""
                a = t[:, :]
                return bass.AP(a.tensor, a.offset + s * FD,
                               [list(a.ap[0]), [FQ, BQ], [1, FQ]])

            def tri(t, s=0, stride=FD):
                """3-entry view [P, 3, BQ, FQ] at slot s."""
                a = t[:, :]
                return bass.AP(a.tensor, a.offset + s * FD,
                               [list(a.ap[0]), [stride, 3], [FQ, BQ],
                                [1, FQ]])

            def six(t):
                a = t[:, :]
                return bass.AP(a.tensor, a.offset,
                               [list(a.ap[0]), [FD, 6], [FQ, BQ], [1, FQ]])

            def bc3(x):
                """broadcast a [P, BQ, FQ] view over 3."""
                return bass.AP(x.tensor, x.offset,
                               [list(x.ap[0]), [0, 3]] +
                               [list(d) for d in x.ap[1:]])

            def bc6(x):
                return bass.AP(x.tensor, x.offset,
                               [list(x.ap[0]), [0, 6]] +
                               [list(d) for d in x.ap[1:]])

            def qv3(w):
                """window triple [P, 3(i), BQ, FQ] of pq for window w."""
                g, slot = win_map[w]
                a = pq[:, :]
                return bass.AP(a.tensor, a.offset + g * GWD + slot,
                               [list(a.ap[0]), [G * GWD, 3],
                                [3 * G * GWD, BQ], [1, FQ]])

            # ---- e_k = q_{n+o_k} - p_n  (f32 windows -> bf16) ----
            Et = wpool.tile([P, 18 * FD], BF, tag="E", uniquify=True,
                            bufs=2)
            eT = lambda k: tri(Et, 3 * k)
            for k in range(K):
                tt(AL.subtract, eT(k), qv3(k + 1), qv3(0))

            # ---- A = sum_k e_k (stab w_k t_k)^T, structured ----
            # col0 = -H0 - wp1 e1 + wp4 e4 + H5 ; col1 = -H0 - wp2 e2
            #      + wp3 e3 + H5 ; col2 = sum_k wz_k e_k ; H- = wp- e-.
            H0 = wt("H0", BF, 3)
            H5 = wt("H5", BF, 3)
            tt(AL.mult, tri(H0), eT(0), r_wp(0))
            tt(AL.mult, tri(H5), eT(5), r_wp(5))
            Ap = wt("Ap", BF, 9)
            t3 = wt("t3", BF, 3)
            col = lambda j: tri(Ap, j, stride=3 * FD)
            tt(AL.mult, col(0), eT(4), r_wp(4))
            tt(AL.add, col(0), col(0), tri(H5))
            tt(AL.subtract, col(0), col(0), tri(H0))
            tt(AL.mult, tri(t3), eT(1), r_wp(1))
            tt(AL.subtract, col(0), col(0), tri(t3))
            tt(AL.mult, col(1), eT(3), r_wp(3))
            tt(AL.add, col(1), col(1), tri(H5))
            tt(AL.subtract, col(1), col(1), tri(H0))
            tt(AL.mult, tri(t3), eT(2), r_wp(2))
            tt(AL.subtract, col(1), col(1), tri(t3))
            tt(AL.mult, col(2), eT(0), r_wz(0))
            for k in range(1, K):
                tt(AL.mult, tri(t3), eT(k), r_wz(k))
                tt(AL.add, col(2), col(2), tri(t3))

            aE = lambda i, j: ent(Ap, i * 3 + j)

            # ---- C = A^T A (6-pack: d0 d1 d2 o01 o02 o12) ----
            sqA = wt("sqA", BF, 9)
            act.square(sqA[:, :], Ap[:, :])
            Cp = wpool.tile([P, 6 * FD], BF, tag="C", uniquify=True, bufs=2)
            tt(AL.add, tri(Cp, 0), tri(sqA, 0), tri(sqA, 3))
            tt(AL.add, tri(Cp, 0), tri(Cp, 0), tri(sqA, 6))
            tmpb = wt("tmpb")
            for s, (a, b) in enumerate(((0, 1), (0, 2), (1, 2))):
                dst = ent(Cp, 3 + s)
                tt(AL.mult, dst, aE(0, a), aE(0, b))
                tt(AL.mult, ent(tmpb), aE(1, a), aE(1, b))
                tt(AL.add, dst, dst, ent(tmpb))
                tt(AL.mult, ent(tmpb), aE(2, a), aE(2, b))
                tt(AL.add, dst, dst, ent(tmpb))
            cE = lambda s: ent(Cp, s)

            # ---- detA (bf16) -> dsg ----
            u0, u1, u2 = wt("u0"), wt("u1"), wt("u2")
            detA = wt("detA")
            tt(AL.mult, ent(u0), aE(1, 1), aE(2, 2))
            tt(AL.mult, ent(tmpb), aE(2, 1), aE(1, 2))
            tt(AL.subtract, ent(u0), ent(u0), ent(tmpb))
            tt(AL.mult, ent(u1), aE(0, 1), aE(2, 2))
            tt(AL.mult, ent(tmpb), aE(2, 1), aE(0, 2))
            tt(AL.subtract, ent(u1), ent(u1), ent(tmpb))
            tt(AL.mult, ent(u2), aE(0, 1), aE(1, 2))
            tt(AL.mult, ent(tmpb), aE(1, 1), aE(0, 2))
            tt(AL.subtract, ent(u2), ent(u2), ent(tmpb))
            tt(AL.mult, ent(detA), aE(0, 0), ent(u0))
            tt(AL.mult, ent(tmpb), aE(1, 0), ent(u1))
            tt(AL.subtract, ent(detA), ent(detA), ent(tmpb))
            tt(AL.mult, ent(tmpb), aE(2, 0), ent(u2))
            tt(AL.add, ent(detA), ent(detA), ent(tmpb))
            dsg = wpool.tile([P, FD], BF, tag="x2", uniquify=True, bufs=8,
                             name="dsg")
            act.sign(dsg[:, :], detA[:, :])

            # ---- invariants (head part) ----
            sqb3 = wpool.tile([P, 3 * FD], BF, tag="sqb", uniquify=True,
                              bufs=2)
            act.square(sqb3[:, :], Cp[:, 3 * FD:6 * FD])
            sq01, sq02, sq12 = ent(sqb3, 0), ent(sqb3, 1), ent(sqb3, 2)
            p1 = wpool.tile([P, FD], BF, tag="x2", uniquify=True, bufs=8,
                            name="p1")
            tt(AL.add, ent(p1), sq01, sq02)
            tt(AL.add, ent(p1), ent(p1), sq12)
            trb = wpool.tile([P, FD], BF, tag="x2", uniquify=True, bufs=8,
                             name="trb")
            tt(AL.add, ent(trb), cE(0), cE(1))
            tt(AL.add, ent(trb), ent(trb), cE(2))
            qm = wpool.tile([P, FD], BF, tag="x2", uniquify=True, bufs=8,
                            name="qm")
            act.mul(qm[:, :], trb[:, :], 1.0 / 3.0)

            yield   # -------- head/tail split --------

            # ---- eigen scalar chain ----
            b3 = wt("b3", BF, 3)
            tt(AL.subtract, tri(b3), tri(Cp, 0), bc3(ent(qm)))
            b0, b1, b2 = ent(b3, 0), ent(b3, 1), ent(b3, 2)
            sb3 = wt("sb3", BF, 3)
            act.square(sb3[:, :], b3[:, :])
            p2 = wt("p2")
            tt(AL.add, ent(p2), ent(sb3, 0), ent(sb3, 1))
            tt(AL.add, ent(p2), ent(p2), ent(sb3, 2))
            vec.scalar_tensor_tensor(out=ent(p2), in0=ent(p1), scalar=2.0,
                                     in1=ent(p2), op0=AL.mult, op1=AL.add)
            vec.tensor_scalar_max(out=p2[:, :], in0=p2[:, :], scalar1=1e-18)
            lnp6 = wt("lnp6", F32)
            act.activation(lnp6[:, :], p2[:, :], AF.Ln, scale=4.0 / 6.0)
            two_p = wt("two_p", F32)
            act.activation(two_p[:, :], lnp6[:, :], AF.Exp, scale=0.5)
            pinv8 = wt("pinv8", F32)
            act.activation(pinv8[:, :], lnp6[:, :], AF.Exp, scale=-1.5)

            # detC with diagonal b0/b1/b2, off-diag C01/C02/C12
            cp01, cp02, cp12 = wt("cp01"), wt("cp02"), wt("cp12")
            tt(AL.mult, ent(cp01), cE(4), cE(5))
            tt(AL.mult, ent(cp02), cE(3), cE(5))
            tt(AL.mult, ent(cp12), cE(3), cE(4))
            ub0, ub1, ub2 = wt("ub0"), wt("ub1"), wt("ub2")
            tt(AL.mult, ent(ub0), b1, b2)
            tt(AL.subtract, ent(ub0), ent(ub0), sq12)
            tt(AL.mult, ent(ub1), cE(3), b2)
            tt(AL.subtract, ent(ub1), ent(ub1), ent(cp01))
            tt(AL.mult, ent(ub2), b1, cE(4))
            tt(AL.subtract, ent(ub2), ent(cp02), ent(ub2))
            detC = wt("detC")
            tt(AL.mult, ent(detC), b0, ent(ub0))
            tt(AL.mult, ent(tmpb), cE(3), ent(ub1))
            tt(AL.subtract, ent(detC), ent(detC), ent(tmpb))
            tt(AL.mult, ent(tmpb), cE(4), ent(ub2))
            tt(AL.add, ent(detC), ent(detC), ent(tmpb))

            r = wt("r", F32)
            vec.scalar_tensor_tensor(out=ent(r), in0=ent(detC), scalar=4.0,
                                     in1=ent(pinv8), op0=AL.mult,
                                     op1=AL.mult)
            vec.tensor_scalar(out=r[:, :], in0=r[:, :], scalar1=RCLAMP,
                              scalar2=-RCLAMP, op0=AL.min, op1=AL.max)
            r2 = wt("r2", F32)
            act.square(r2[:, :], r[:, :])
            lnomr = wt("lnomr", F32)
            act.activation(lnomr[:, :], r2[:, :], AF.Ln, bias=1.0,
                           scale=-1.0)
            eh = wt("eh", F32)
            act.activation(eh[:, :], lnomr[:, :], AF.Exp, scale=-0.5)
            s_ = wt("s_", F32)
            tt(AL.mult, ent(s_), ent(r), ent(eh))
            at = wt("at", F32)
            act.activation(at[:, :], s_[:, :], AF.Arctan)
            sinL, sinM = wt("sinL"), wt("sinM")
            act.activation(sinL[:, :], at[:, :], AF.Sin,
                           bias=bias_sinl[:, :], scale=-1.0 / 3.0)
            act.activation(sinM[:, :], at[:, :], AF.Sin, scale=-1.0 / 3.0)
            two_pb = wt("two_pb")
            act.copy(two_pb[:, :], two_p[:, :])
            lam3, lam2, lam1 = wt("lam3"), wt("lam2"), wt("lam1")
            tt(AL.mult, ent(tmpb), ent(two_pb), ent(sinL))
            tt(AL.add, ent(lam3), ent(qm), ent(tmpb))
            tt(AL.mult, ent(tmpb), ent(two_pb), ent(sinM))
            tt(AL.add, ent(lam2), ent(qm), ent(tmpb))
            tt(AL.subtract, ent(tmpb), ent(trb), ent(lam3))
            tt(AL.subtract, ent(lam1), ent(tmpb), ent(lam2))
            d32 = wt("d32")
            tt(AL.subtract, ent(tmpb), ent(sinL), ent(sinM))
            tt(AL.mult, ent(d32), ent(two_pb), ent(tmpb))
            d21, d31 = wt("d21"), wt("d31")
            tt(AL.subtract, ent(d21), ent(lam2), ent(lam1))
            tt(AL.subtract, ent(d31), ent(lam3), ent(lam1))
            l2c, l3c = wt("l2c"), wt("l3c")
            vec.tensor_scalar_max(out=l2c[:, :], in0=lam2[:, :],
                                  scalar1=CLIPV)
            vec.tensor_scalar_max(out=l3c[:, :], in0=lam3[:, :],
                                  scalar1=CLIPV)
            g2, g3 = wt("g2"), wt("g3")
            tmpf = wt("tmpf", F32)
            act.activation(tmpf[:, :], l2c[:, :], AF.Ln)
            act.activation(g2[:, :], tmpf[:, :], AF.Exp, scale=-0.5)
            act.activation(tmpf[:, :], l3c[:, :], AF.Ln)
            act.activation(g3[:, :], tmpf[:, :], AF.Exp, scale=-0.5)
            l3sq = wt("l3sq")
            act.square(l3sq[:, :], l3c[:, :])

            tmpb2 = wt("tmpb2")

            def safe_recip(dst, x):
                """dst = sign(x)/max(|x|, 1e-6*l3sq)."""
                act.activation(tmpb2[:, :], x[:, :], AF.Abs)
                vec.scalar_tensor_tensor(out=ent(tmpb2), in0=ent(l3sq),
                                         scalar=1e-6, in1=ent(tmpb2),
                                         op0=AL.mult, op1=AL.max)
                act.activation(tmpf[:, :], tmpb2[:, :], AF.Ln)
                act.activation(dst[:, :], tmpf[:, :], AF.Exp, scale=-1.0)
                act.sign(tmpb2[:, :], x[:, :])
                tt(AL.mult, ent(dst), ent(dst), ent(tmpb2))

            den2m, den3 = wt("den2m"), wt("den3")
            tt(AL.mult, ent(den2m), ent(d21), ent(d32))
            tt(AL.mult, ent(den3), ent(d31), ent(d32))
            inv2m, inv3 = wt("inv2m"), wt("inv3")
            safe_recip(inv2m, den2m)
            safe_recip(inv3, den3)
            gam2, gam3 = wt("gam2"), wt("gam3")
            vec.scalar_tensor_tensor(out=ent(gam2), in0=ent(g2), scalar=-1.0,
                                     in1=ent(inv2m), op0=AL.mult,
                                     op1=AL.mult)
            tt(AL.mult, ent(gam3), ent(g3), ent(inv3))
            c4p, c3, c4 = wt("c4p"), wt("c3"), wt("c4")
            tt(AL.mult, ent(c4p), ent(dsg), ent(gam3))
            tt(AL.add, ent(c3), ent(gam2), ent(c4p))
            tt(AL.mult, ent(c4), ent(c4p), ent(d32))

            # ---- N1 = C - lam1, Mdiag = Cdiag - lam3, T2 = N1 (C-lam3) ----
            N1p = wt("N1p", BF, 6)
            tt(AL.subtract, tri(N1p, 0), tri(Cp, 0), bc3(ent(lam1)))
            vec.tensor_copy(N1p[:, 3 * FD:6 * FD], Cp[:, 3 * FD:6 * FD])
            Md3 = wt("Md3", BF, 3)
            tt(AL.subtract, tri(Md3), tri(Cp, 0), bc3(ent(lam3)))
            T2p = wt("T2p", BF, 6)
            tt(AL.mult, tri(T2p, 0), tri(N1p, 0), tri(Md3))
            tt(AL.add, ent(T2p, 0), ent(T2p, 0), sq01)
            tt(AL.add, ent(T2p, 0), ent(T2p, 0), sq02)
            tt(AL.add, ent(T2p, 1), ent(T2p, 1), sq01)
            tt(AL.add, ent(T2p, 1), ent(T2p, 1), sq12)
            tt(AL.add, ent(T2p, 2), ent(T2p, 2), sq02)
            tt(AL.add, ent(T2p, 2), ent(T2p, 2), sq12)
            tq = wt("tq")
            # (0,1): C01*(Nd0+Md1) + cp01
            tt(AL.add, ent(tq), ent(N1p, 0), ent(Md3, 1))
            tt(AL.mult, ent(T2p, 3), cE(3), ent(tq))
            tt(AL.add, ent(T2p, 3), ent(T2p, 3), ent(cp01))
            # (0,2): C02*(Nd0+Md2) + cp02
            tt(AL.add, ent(tq), ent(N1p, 0), ent(Md3, 2))
            tt(AL.mult, ent(T2p, 4), cE(4), ent(tq))
            tt(AL.add, ent(T2p, 4), ent(T2p, 4), ent(cp02))
            # (1,2): C12*(Nd1+Md2) + cp12
            tt(AL.add, ent(tq), ent(N1p, 1), ent(Md3, 2))
            tt(AL.mult, ent(T2p, 5), cE(5), ent(tq))
            tt(AL.add, ent(T2p, 5), ent(T2p, 5), ent(cp12))

            # ---- W2 = c3*T2 + c4*N1 (in place on T2p) ----
            t6 = wt("t6", BF, 6)
            tt(AL.mult, six(t6), six(N1p), bc6(ent(c4)))
            tt(AL.mult, six(T2p), six(T2p), bc6(ent(c3)))
            tt(AL.add, six(T2p), six(T2p), six(t6))
            SYM = {(0, 0): 0, (1, 1): 1, (2, 2): 2,
                   (0, 1): 3, (1, 0): 3, (0, 2): 4, (2, 0): 4,
                   (1, 2): 5, (2, 1): 5}
            w2 = lambda c, j: bc3(ent(T2p, SYM[(c, j)]))

            # ---- Y = A @ W2 ----
            Yp = wt("Yp", BF, 9)
            acol = lambda c: tri(Ap, c, stride=3 * FD)
            ycol = lambda j: tri(Yp, j, stride=3 * FD)
            for j in range(3):
                tt(AL.mult, ycol(j), acol(0), w2(0, j))
                for c in (1, 2):
                    tt(AL.mult, tri(t3), acol(c), w2(c, j))
                    tt(AL.add, ycol(j), ycol(j), tri(t3))

            # ---- R = Y + cof(Y) ----
            Rp = wt("Rp", BF, 9)
            yE = lambda i, j: ent(Yp, i * 3 + j)
            cf = wt("cf")
            for i in range(3):
                for j in range(3):
                    i1, i2 = (i + 1) % 3, (i + 2) % 3
                    j1, j2 = (j + 1) % 3, (j + 2) % 3
                    tt(AL.mult, ent(cf), yE(i1, j1), yE(i2, j2))
                    tt(AL.mult, ent(tmpb), yE(i1, j2), yE(i2, j1))
                    tt(AL.subtract, ent(cf), ent(cf), ent(tmpb))
                    tt(AL.add, ent(Rp, i * 3 + j), yE(i, j), ent(cf))

            # ---- energy ----
            rcol = lambda j: tri(Rp, j, stride=3 * FD)
            Rpm3 = wt("Rpm3", BF, 3)
            tt(AL.add, tri(Rpm3), rcol(0), rcol(1))
            Z3 = wt("Z3", BF, 3)
            dfc3 = wt("dfc3", BF, 3)
            sqd3 = wt("sqd3", BF, 3)
            ns6 = wt("ns6", BF, 6)
            combos = ((AL.add, tri(Rpm3)), (AL.add, rcol(0)),
                      (AL.add, rcol(1)), (AL.subtract, rcol(1)),
                      (AL.subtract, rcol(0)), (AL.subtract, tri(Rpm3)))
            for k in range(K):
                tt(AL.mult, tri(Z3), rcol(2), r_tz(k))
                tt(AL.subtract, tri(dfc3), eT(k), tri(Z3))
                op, cv = combos[k]
                tt(op, tri(dfc3), tri(dfc3), cv)
                act.square(sqd3[:, :], dfc3[:, :])
                tt(AL.add, ent(ns6, k), ent(sqd3, 0), ent(sqd3, 1))
                tt(AL.add, ent(ns6, k), ent(ns6, k), ent(sqd3, 2))
            act.activation(ns6[:, :], ns6[:, :], AF.Sqrt)
            tt(AL.mult, six(ns6), six(ns6), wrow6())
            s3 = wt("s3", BF, 3)
            tt(AL.add, tri(s3), tri(ns6, 0), tri(ns6, 3))
            nrg = wt("nrg")
            tt(AL.add, ent(nrg), ent(s3, 0), ent(s3, 1))
            tt(AL.add, ent(nrg), ent(nrg), ent(s3, 2))
            vec.tensor_scalar_min(out=nrg[:, :], in0=nrg[:, :], scalar1=1.0)
            vec.tensor_reduce(out=outacc[:, qb * BQ:(qb + 1) * BQ],
                              in_=ent(nrg), axis=mybir.AxisListType.X,
                              op=AL.add)

        gens = [pass_gen(qb) for qb in range(NQ)]
        next(gens[0])
        for qb in range(1, NQ):
            next(gens[qb])
            for _ in gens[qb - 1]:
                pass
        for _ in gens[NQ - 1]:
            pass

        nc.sync.dma_start(out_d[:, :], outacc[:, :])

    nc.compile()
    return nc


def _get_nc(wingeo):
    if wingeo not in _nc_cache:
        _nc_cache[wingeo] = _build_nc(wingeo)
    return _nc_cache[wingeo]


# ---------------------------------------------------------------------------
# Entry point
# ---------------------------------------------------------------------------

def _install_ntff_shim():
    """Provide antenv.axon_hooks (missing in this image) so
    run_bass_kernel_spmd(trace=True) can reach the NTFF profiler in
    libaxon_pjrt.so."""
    import types

    try:
        import antenv.axon_hooks  # noqa: F401
        return True
    except ImportError:
        pass
    try:
        import antenv
        from trn_agent_boot.trn_boot import _ntff_profile_via_ctypes
    except ImportError:
        return False
    mod = types.ModuleType("antenv.axon_hooks")
    state = {"hook": None}
    mod.set_axon_ntff_profile_hook = lambda h: state.__setitem__("hook", h)
    mod.get_axon_ntff_profile_hook = lambda: state["hook"]
    sys.modules["antenv.axon_hooks"] = mod
    antenv.axon_hooks = mod
    try:
        hook = _ntff_profile_via_ctypes("/opt/axon/libaxon_pjrt.so")
    except OSError:
        hook = None
    if hook is not None:
        mod.set_axon_ntff_profile_hook(hook)
    return hook is not None


def kernel(**inputs) -> np.ndarray:
    pred = np.asarray(inputs["prediction"], np.float32)
    adj_idx = np.asarray(inputs["adj_list_indices"])
    adj_w = np.asarray(inputs["adj_list_weights"], np.float32)
    tev_T = np.asarray(inputs["template_edge_vectors_T"], np.float32)

    wk, tzk = _build_offset_classes(adj_idx, adj_w, tev_T)
    in_maps, wingeo = _host_prepare(pred, wk, tzk)

    nc = _get_nc(wingeo)
    import os
    trace = bool(int(os.environ.get("ARAP_TRACE", "0")))
    if trace:
        trace = _install_ntff_shim()
    try:
        res = run_bass_kernel_spmd(nc, in_maps, core_ids=list(range(NCORES)),
                                   trace=trace)
    except Exception:
        if not trace:
            raise
        res = run_bass_kernel_spmd(nc, in_maps, core_ids=list(range(NCORES)),
                                   trace=False)
    kernel._last_exec_ns = res.exec_time_ns
    kernel._last_results = res

    total = np.zeros(B, np.float64)
    for c in range(NCORES):
        total += res.results[c]["out"].astype(np.float64).sum(axis=0)
    return (total / NV).astype(np.float32)


kernel._last_exec_ns = None
